# revision 22
# baseline (speedup 1.0000x reference)
"""TRN2 Bass kernel for causal multi-head attention with RoPE (v3).

Problem: B=2, S=2048, HID=2048, NH=16, HD=128 (fp32 reference).
Sharding: 8 cores = 2 (batch) x 4 (head-groups of 4 heads).
Each core computes q/k/v projections for its 4 heads (column-parallel),
RoPE, causal attention, and a row-parallel partial o_proj; the host sums
the 4 partials per batch.

v3 design: all matmul operands bf16 (PSUM accumulates fp32), everything
SBUF-resident (no DRAM spill), and one continuous PE stream where the
attention tiles of chunk c are interleaved with "filler" matmuls --
o_proj(c-1) and the q/k/v projections of chunk c+1.  The filler PE work
hides the ACT exp latency of the attention softmax, so neither engine
gates: the kernel runs at the tensor-engine roofline end to end and the
HAM clock never re-throttles.  PSUM: acc(2) + scores(3) + attn-out(2) +
softmax-sum(1) = 8 banks.
"""
import os
import sys

if "/opt/trn_rl_repo" not in sys.path:
    sys.path.insert(0, "/opt/trn_rl_repo")

import numpy as np
import ml_dtypes

import concourse.bass as bass
import concourse.bass_isa as bass_isa
import concourse.mybir as mybir
import concourse.tile as tile
from concourse import bacc
from concourse.bass_utils import run_bass_kernel_spmd
from contextlib import ExitStack

P = 128
B, S, HID, NH = 2, 2048, 2048, 16
HD = HID // NH              # 128
H = 4                       # heads per core
DPC = H * HD                # 512 dims per core
KO = HID // P               # 16 contraction chunks
SC = S // 512               # 4 seq chunks of 512
ST = S // P                 # 16 seq tiles of 128
SCALE = 1.0 / float(np.sqrt(HD))
LOOK = 2                    # attention pipeline lookahead (tiles)

f32 = mybir.dt.float32
bf16 = mybir.dt.bfloat16

_CACHED_NC = None


def build_nc():
    AF = mybir.ActivationFunctionType
    nc = bacc.Bacc(None, target_bir_lowering=False)

    xt = nc.declare_dram_parameter("xt", [P, KO, S], bf16, isOutput=False)
    wq = nc.declare_dram_parameter("wq", [H, P, KO, HD], bf16, isOutput=False)
    wk = nc.declare_dram_parameter("wk", [H, P, KO, HD], bf16, isOutput=False)
    wv = nc.declare_dram_parameter("wv", [P, KO, DPC], bf16, isOutput=False)
    wo = nc.declare_dram_parameter("wo", [P, H, HID], bf16, isOutput=False)
    cosf = nc.declare_dram_parameter("cosf", [P, S], f32, isOutput=False)
    sinf = nc.declare_dram_parameter("sinf", [P, S], f32, isOutput=False)
    bmask = nc.declare_dram_parameter("bmask", [P, 4, 512], bf16, isOutput=False)
    out_p = nc.declare_dram_parameter("out_p", [S, HID], f32, isOutput=True)

    out3 = out_p.rearrange("(st p) n -> p st n", p=P)

    with tile.TileContext(nc) as tc:
        with ExitStack() as top:
            const = top.enter_context(tc.tile_pool(name="const", bufs=1))
            wpool = top.enter_context(tc.tile_pool(name="wpool", bufs=1))
            kvpool = top.enter_context(tc.tile_pool(name="kv", bufs=1))
            xpool = top.enter_context(tc.tile_pool(name="xp", bufs=2))
            qpool = top.enter_context(tc.tile_pool(name="qp", bufs=2))
            aopool = top.enter_context(tc.tile_pool(name="ao", bufs=3))
            rtmp = top.enter_context(tc.tile_pool(name="rt", bufs=2))
            ppool = top.enter_context(tc.tile_pool(name="pp", bufs=8))
            ost = top.enter_context(tc.tile_pool(name="ost", bufs=5))
            stage = top.enter_context(tc.tile_pool(name="stage", bufs=2))
            stg1 = top.enter_context(tc.tile_pool(name="stg1", bufs=1))
            # PSUM: exactly 8 banks
            acc = top.enter_context(tc.tile_pool(name="acc", bufs=2, space="PSUM"))
            sps = top.enter_context(tc.tile_pool(name="sps", bufs=3, space="PSUM"))
            obp = top.enter_context(tc.tile_pool(name="obp", bufs=2, space="PSUM"))
            smp = top.enter_context(tc.tile_pool(name="smp", bufs=1, space="PSUM"))

            # ---- static tiles ----
            wvs = wpool.tile([P, KO, DPC], bf16)
            wqs = wpool.tile([P, H, KO, HD], bf16)
            wks = wpool.tile([P, H, KO, HD], bf16)
            wot = wpool.tile([P, H, HID], bf16)
            cosT = const.tile([P, S], f32)
            sinT = const.tile([P, S], f32)
            bmt = const.tile([P, 4, 512], bf16)
            zb = const.tile([P, 1], f32)
            ones_col = const.tile([P, 1], bf16)
            kt = kvpool.tile([P, H, S], bf16)        # K^T, RoPE'd, all chunks
            vsb = kvpool.tile([P, ST, H, HD], bf16)  # V natural layout

            nc.vector.memset(zb[:], 0.0)
            nc.vector.memset(ones_col[:], 1.0)
            # fixed softmax-sum bank: only rows 0/32/64/96 are ever written
            # (by the col-group sum matmuls); the rest stays zero so the
            # 128-channel all-reduce combine sees sum + zeros
            sm_fx = smp.tile([P, 512], f32, tag="sm", name="sm")
            nc.vector.memset(sm_fx[:], 0.0)

            # ---- load order tuned for the ko-pipelined chunk-0 V-proj:
            # wv ko-quarters and per-ko x0 slices interleaved on both queues,
            # then wk (sync) / cos+sin+wq (scalar), wot last ----
            xs_c = [xpool.tile([P, KO, 512], bf16, tag="xs", name=f"xs{c}")
                    for c in range(SC)]
            nc.sync.dma_start(wvs[:, 0:4], wv[:, 0:4])
            nc.scalar.dma_start(wvs[:, 4:8], wv[:, 4:8])
            nc.gpsimd.dma_start(xs_c[0][:, 0], xt[:, 0, 0:512])
            nc.gpsimd.dma_start(xs_c[0][:, 1], xt[:, 1, 0:512])
            nc.sync.dma_start(wvs[:, 8:12], wv[:, 8:12])
            nc.scalar.dma_start(wvs[:, 12:16], wv[:, 12:16])
            for ko in range(2, KO):
                eng = (nc.sync, nc.scalar, nc.gpsimd)[ko % 3]
                eng.dma_start(xs_c[0][:, ko], xt[:, ko, 0:512])
            for h in range(H):
                nc.sync.dma_start(wks[:, h], wk[h])
            nc.scalar.dma_start(cosT[:], cosf[:])
            nc.scalar.dma_start(sinT[:], sinf[:])
            for h in range(H):
                nc.scalar.dma_start(wqs[:, h], wq[h])
            nc.scalar.dma_start(bmt[:], bmask[:])
            nc.sync.dma_start(wot[:], wo[:])

            qt_c = [None] * SC
            aot_c = [None] * SC

            # ---------- filler item builders (each item: emit ~1 matmul) ----
            def vproj_items(sc):
                items = []
                box = {}
                for st4 in range(4):
                    st = sc * 4 + st4
                    for ko in range(KO):
                        def mk(st=st, st4=st4, ko=ko):
                            if ko == 0:
                                box['ps'] = acc.tile([P, DPC], f32, tag="acc", name="vps")
                            nc.tensor.matmul(
                                box['ps'][:],
                                xs_c[sc][:, ko, st4 * P:(st4 + 1) * P],
                                wvs[:, ko],
                                start=(ko == 0),
                                stop=(ko == KO - 1),
                            )
                            if ko == KO - 1:
                                nc.vector.tensor_copy(
                                    vsb[:, st],
                                    box['ps'].rearrange("p (h d) -> p h d", h=H),
                                )
                        items.append(mk)
                return items

            def qkproj_items(sc, ws, dst, dst_sl):
                # dst[dst_sl(h)] <- RoPE(ws[h].T @ x_chunk) in bf16
                ssl = slice(sc * 512, (sc + 1) * 512)
                items = []
                box = {}
                for h in range(H):
                    for ko in range(KO):
                        def mk(h=h, ko=ko):
                            if ko == 0:
                                box['ps'] = acc.tile([P, 512], f32, tag="acc", name="qkps")
                            ps = box['ps']
                            nc.tensor.matmul(
                                ps[:],
                                ws[:, h, ko],
                                xs_c[sc][:, ko],
                                start=(ko == 0),
                                stop=(ko == KO - 1),
                            )
                            if ko == KO - 1:
                                # RoPE eviction; sinT pre-signed (-sin top)
                                t0 = rtmp.tile([P, 512], f32, tag="t0", name="t0")
                                c0 = rtmp.tile([P, 512], f32, tag="c0", name="c0")
                                nc.vector.tensor_mul(
                                    t0[0:64], ps[64:128], sinT[0:64, ssl])
                                nc.vector.tensor_mul(
                                    t0[64:128], ps[0:64], sinT[64:128, ssl])
                                nc.vector.tensor_mul(c0[:], ps[:], cosT[:, ssl])
                                nc.vector.tensor_add(dst[dst_sl(h)], c0[:], t0[:])
                        items.append(mk)
                return items

            def oproj_items(cc, alt_out=False):
                aot = aot_c[cc]
                items = []
                box = {}
                for st4 in range(4):
                    st = cc * 4 + st4
                    for nch in range(4):
                        for dc in range(H):
                            def mk(st=st, st4=st4, nch=nch, dc=dc):
                                if dc == 0:
                                    box['ps'] = acc.tile([P, 512], f32, tag="acc", name="ops")
                                pso = box['ps']
                                nc.tensor.matmul(
                                    pso[:],
                                    aot[:, dc, st4 * P:(st4 + 1) * P],
                                    wot[:, dc, nch * 512:(nch + 1) * 512],
                                    start=(dc == 0),
                                    stop=(dc == H - 1),
                                )
                                if dc == H - 1:
                                    ob2 = ost.tile([P, 512], f32, tag="ob2", name="ob2")
                                    nc.vector.tensor_copy(ob2[:], pso[:])
                                    eng = nc.scalar if (alt_out and (st4 * 4 + nch) % 2) else nc.sync
                                    eng.dma_start(
                                        out3[:, st, nch * 512:(nch + 1) * 512],
                                        ob2[:],
                                    )
                            items.append(mk)
                return items

            def mix_filler(a, b):
                """Proportionally merge list b into list a."""
                if not a:
                    return list(b)
                if not b:
                    return list(a)
                out = []
                ia = ib = 0
                while ia < len(a) or ib < len(b):
                    if ib * len(a) <= ia * len(b) and ib < len(b):
                        out.append(b[ib]); ib += 1
                    elif ia < len(a):
                        out.append(a[ia]); ia += 1
                    else:
                        out.append(b[ib]); ib += 1
                return out

            # ---------- attention ----------
            def attention_tiles(sc):
                """Return a list of per-tile emitters; each emits scores+exp
                (+mask), appends to a pend queue, and flushes P@V/sum for the
                tile LOOK back (plus the normalize chain at head end)."""
                nt = 4 * (sc + 1)
                qc = qt_c[sc]
                aot = aot_c[sc]
                # dense tiles first: every sum col-group's start=True write is
                # then full-width [0:512], so no stale PSUM columns survive
                # into the 4-row combine.  sc==0 is all-diagonal: plain
                # per-tile sums into row 0 there.
                t_order = list(range(0, 4 * sc)) + list(range(4 * sc, nt))
                pend = []
                sq = []

                def flush_one():
                    pt, off, ti, t, h, ob, sm = pend.pop(0)
                    nc.tensor.matmul(
                        ob[:, off:512], vsb[:, t, h], pt[:, off:512],
                        start=(ti == 0), stop=(ti == nt - 1),
                    )
                    if sc == 0:
                        nc.tensor.matmul(
                            sm[0:1, off:512], ones_col[:], pt[:, off:512],
                            start=(ti == 0), stop=(ti == nt - 1),
                        )
                    else:
                        sq.append((pt, off, ti))
                        if len(sq) == 4:
                            # softmax sums: 4 concurrent M=1 matmuls in
                            # distinct 32-aligned column groups of the PE
                            # array
                            for (qpt, qoff, qti) in sq:
                                j = qti % 4
                                nc.tensor.matmul(
                                    sm[32 * j:32 * j + 1, qoff:512],
                                    ones_col[:], qpt[:, qoff:512],
                                    start=(qti < 4), stop=(qti >= nt - 4),
                                    tile_position=(0, 32 * j),
                                )
                            sq.clear()
                    if ti == nt - 1:
                        # combine rows 0/32/64/96 (rest of the bank is zero)
                        # via a 128-channel all-reduce, which also serves as
                        # the partition broadcast of 1/Z
                        zsb = stg1.tile([P, 512], f32, tag="zsb", name="zsb")
                        nc.vector.tensor_copy(zsb[:], sm[:])
                        zr = stg1.tile([P, 512], f32, tag="zr", name="zr")
                        nc.gpsimd.partition_all_reduce(
                            zr[:], zsb[:], channels=P,
                            reduce_op=bass_isa.ReduceOp.add)
                        zrcp = stg1.tile([P, 512], f32, tag="zrcp",
                                         name="zrcp")
                        nc.vector.reciprocal_approx_fast(zrcp[:], zr[:])
                        nc.vector.tensor_mul(aot[:, h], ob[:], zrcp[:])

                items = []
                box = {}
                for h in range(H):
                    for ti, t in enumerate(t_order):
                        def mk(h=h, ti=ti, t=t):
                            if ti == 0:
                                box['ob'] = obp.tile([P, 512], f32, tag="ob", name="ob")
                                box['sm'] = sm_fx
                            r = t - 4 * sc
                            off = P * max(r, 0)
                            ps = sps.tile([P, 512], f32, tag="s", name="sco")
                            nc.tensor.matmul(
                                ps[:, off:512],
                                kt[:, h, t * P:(t + 1) * P],
                                qc[:, h, off:512],
                                start=True, stop=True,
                            )
                            pt = ppool.tile([P, 512], bf16, tag="pt", name="pt")
                            nc.scalar.activation(
                                pt[:, off:512], ps[:, off:512], AF.Exp,
                                bias=zb[:], scale=SCALE,
                            )
                            if r >= 0:
                                nc.vector.tensor_mul(
                                    pt[:, off:512], pt[:, off:512],
                                    bmt[:, r, off:512],
                                )
                            pend.append(
                                (pt, off, ti, t, h, box['ob'], box['sm']))
                            if len(pend) > LOOK:
                                flush_one()
                        items.append(mk)

                def drain():
                    while pend:
                        flush_one()
                return items, drain

            # ---------- main schedule ----------
            for sc in range(SC):
                # prefetch next x chunk; NOT on the scalar queue -- the
                # scalar engine must stay dedicated to the exp stream
                # (a DMA_DIRECT2D there stalls behind dozens of ACTIVATEs)
                if sc + 1 < SC:
                    for ko in range(KO):
                        eng = nc.sync if ko % 2 == 0 else nc.gpsimd
                        eng.dma_start(
                            xs_c[sc + 1][:, ko],
                            xt[:, ko, (sc + 1) * 512:(sc + 2) * 512],
                        )
                aot_c[sc] = aopool.tile([P, H, 512], bf16, tag="aot",
                                        name=f"aot{sc}")
                if sc == 0:
                    # no preceding attention to interleave with: emit densely
                    qt_c[0] = qpool.tile([P, H, 512], bf16, tag="qt", name="qt0")
                    for it in vproj_items(0):
                        it()
                    for it in qkproj_items(
                            0, wks, kt,
                            lambda h: (slice(None), h, slice(0, 512))):
                        it()
                    for it in qkproj_items(
                            0, wqs, qt_c[0],
                            lambda h: (slice(None), h, slice(None))):
                        it()

                # filler: o_proj work is deferred TWO chunks so the last
                # chunk (which has no next-chunk projections) still gets two
                # o_proj chunks' worth of PE filler to hide exp latency.
                # o_proj items are spread proportionally among proj items so
                # their output DMAs don't burst the sync queue.
                oproj_f = []
                proj_f = []
                if sc >= 2:
                    oproj_f += oproj_items(sc - 2)
                if sc == SC - 1:
                    oproj_f += oproj_items(sc - 1, alt_out=True)
                if sc + 1 < SC:
                    nsc = sc + 1
                    qt_c[nsc] = qpool.tile([P, H, 512], bf16, tag="qt",
                                           name=f"qt{nsc}")
                    proj_f += vproj_items(nsc)
                    proj_f += qkproj_items(
                        nsc, wks, kt,
                        lambda h, _s=nsc: (slice(None), h,
                                           slice(_s * 512, (_s + 1) * 512)))
                    proj_f += qkproj_items(
                        nsc, wqs, qt_c[nsc],
                        lambda h: (slice(None), h, slice(None)))
                filler = mix_filler(proj_f, oproj_f)

                tiles, drain = attention_tiles(sc)
                # delay filler that reads the just-prefetched x chunk by a
                # few tiles so the DMA has time to land
                lead = 6 if (proj_f and sc >= 1) else 0
                step = len(filler) / max(1, len(tiles) - lead)
                fi_target = -lead * step
                fi = 0
                for it in tiles:
                    it()
                    fi_target += step
                    while fi < int(fi_target):
                        filler[fi]()
                        fi += 1
                drain()
                while fi < len(filler):
                    filler[fi]()
                    fi += 1

            for it in oproj_items(SC - 1):
                it()

    nc.compile()
    return nc


def _host_prep(hidden_states, position_ids, Wq, Wk, Wv, Wo):
    """Build the 8 per-core input maps (bf16 operands)."""
    inv_freq = 1.0 / (10000.0 ** (np.arange(0, HD, 2, dtype=np.float32) / HD))
    t = np.arange(S, dtype=np.float32)
    freqs = np.outer(t, inv_freq).astype(np.float32)  # [S, 64]

    bm = np.empty((P, 4, 512), dtype=np.float32)
    i = np.arange(P)[:, None, None]
    r = np.arange(4)[None, :, None]
    j = np.arange(512)[None, None, :]
    bm[:] = np.where(i + P * r <= j, 1.0, 0.0)
    bm = bm.astype(ml_dtypes.bfloat16)

    in_maps = []
    per_batch = []
    for b in range(B):
        xT = np.ascontiguousarray(hidden_states[b].T)  # [HID, S]
        xt_sw = np.ascontiguousarray(
            xT.reshape(KO, P, S).transpose(1, 0, 2)
        ).astype(ml_dtypes.bfloat16)  # [P, KO, S]
        fp = freqs[position_ids[b]]  # [S, 64]
        ch = np.cos(fp).T            # [64, S]
        sh = np.sin(fp).T
        cosf = np.ascontiguousarray(np.concatenate([ch, ch], axis=0))   # [128, S]
        sinf = np.ascontiguousarray(np.concatenate([-sh, sh], axis=0))  # signed
        per_batch.append((xt_sw, cosf, sinf))

    for core in range(8):
        b, hg = core // 4, core % 4
        sl = slice(hg * DPC, (hg + 1) * DPC)
        xt_sw, cosf, sinf = per_batch[b]
        wq_sw = np.ascontiguousarray(
            Wq[sl].T.reshape(KO, P, H, HD).transpose(2, 1, 0, 3)
        ).astype(ml_dtypes.bfloat16)  # [H, P, KO, HD]
        wk_sw = np.ascontiguousarray(
            Wk[sl].T.reshape(KO, P, H, HD).transpose(2, 1, 0, 3)
        ).astype(ml_dtypes.bfloat16)
        wv_sw = np.ascontiguousarray(
            Wv[sl].T.reshape(KO, P, DPC).transpose(1, 0, 2)
        ).astype(ml_dtypes.bfloat16)  # [P, KO, DPC]
        wo_sw = np.ascontiguousarray(
            Wo[:, sl].T.reshape(H, HD, HID).transpose(1, 0, 2)
        ).astype(ml_dtypes.bfloat16)  # [P, H, HID]
        in_maps.append({
            "xt": xt_sw, "wq": wq_sw, "wk": wk_sw, "wv": wv_sw, "wo": wo_sw,
            "cosf": cosf, "sinf": sinf, "bmask": bm,
        })
    return in_maps


def kernel(hidden_states, attention_mask, position_ids, Wq, Wk, Wv, Wo,
           _trace=False, _trace_kwargs=None):
    global _CACHED_NC
    hidden_states = np.asarray(hidden_states, dtype=np.float32)
    position_ids = np.asarray(position_ids)
    Wq, Wk, Wv, Wo = (np.asarray(w, dtype=np.float32) for w in (Wq, Wk, Wv, Wo))

    if _CACHED_NC is None:
        _CACHED_NC = build_nc()
    nc = _CACHED_NC

    in_maps = _host_prep(hidden_states, position_ids, Wq, Wk, Wv, Wo)
    res = run_bass_kernel_spmd(
        nc, in_maps, list(range(8)), trace=_trace, **(_trace_kwargs or {})
    )

    out = np.empty((B, S, HID), dtype=np.float32)
    for b in range(B):
        acc = res.results[b * 4]["out_p"].astype(np.float32)
        for hg in range(1, 4):
            acc = acc + res.results[b * 4 + hg]["out_p"]
        out[b] = acc
    if _trace:
        return out, res
    return out


# revision 25
# speedup vs baseline: 1.0617x; 1.0617x over previous
"""TRN2 Bass kernel for causal multi-head attention with RoPE (v3).

Problem: B=2, S=2048, HID=2048, NH=16, HD=128 (fp32 reference).
Sharding: 8 cores = 2 (batch) x 4 (head-groups of 4 heads).
Each core computes q/k/v projections for its 4 heads (column-parallel),
RoPE, causal attention, and a row-parallel partial o_proj; the host sums
the 4 partials per batch.

v3 design: all matmul operands bf16 (PSUM accumulates fp32), everything
SBUF-resident (no DRAM spill), and one continuous PE stream where the
attention tiles of chunk c are interleaved with "filler" matmuls --
o_proj(c-1) and the q/k/v projections of chunk c+1.  The filler PE work
hides the ACT exp latency of the attention softmax, so neither engine
gates: the kernel runs at the tensor-engine roofline end to end and the
HAM clock never re-throttles.  PSUM: acc(2) + scores(3) + attn-out(2) +
softmax-sum(1) = 8 banks.
"""
import os
import sys

if "/opt/trn_rl_repo" not in sys.path:
    sys.path.insert(0, "/opt/trn_rl_repo")

import numpy as np
import ml_dtypes

import concourse.bass as bass
import concourse.bass_isa as bass_isa
import concourse.mybir as mybir
import concourse.tile as tile
from concourse import bacc
from concourse.bass_utils import run_bass_kernel_spmd
from contextlib import ExitStack

P = 128
B, S, HID, NH = 2, 2048, 2048, 16
HD = HID // NH              # 128
H = 4                       # heads per core
DPC = H * HD                # 512 dims per core
KO = HID // P               # 16 contraction chunks
SC = S // 512               # 4 seq chunks of 512
ST = S // P                 # 16 seq tiles of 128
SCALE = 1.0 / float(np.sqrt(HD))
LOOK = 2                    # attention pipeline lookahead (tiles)

f32 = mybir.dt.float32
bf16 = mybir.dt.bfloat16

_CACHED_NC = None


def build_nc():
    AF = mybir.ActivationFunctionType
    nc = bacc.Bacc(None, target_bir_lowering=False)

    xt = nc.declare_dram_parameter("xt", [P, KO, S], bf16, isOutput=False)
    wq = nc.declare_dram_parameter("wq", [H, P, KO, HD], bf16, isOutput=False)
    wk = nc.declare_dram_parameter("wk", [H, P, KO, HD], bf16, isOutput=False)
    wv = nc.declare_dram_parameter("wv", [P, KO, DPC], bf16, isOutput=False)
    wo = nc.declare_dram_parameter("wo", [P, H, HID], bf16, isOutput=False)
    cosf = nc.declare_dram_parameter("cosf", [P, S], f32, isOutput=False)
    sinf = nc.declare_dram_parameter("sinf", [P, S], f32, isOutput=False)
    bmask = nc.declare_dram_parameter("bmask", [P, 4, 512], bf16, isOutput=False)
    out_p = nc.declare_dram_parameter("out_p", [S, HID], f32, isOutput=True)

    out3 = out_p.rearrange("(st p) n -> p st n", p=P)

    with tile.TileContext(nc) as tc:
        with ExitStack() as top:
            const = top.enter_context(tc.tile_pool(name="const", bufs=1))
            wpool = top.enter_context(tc.tile_pool(name="wpool", bufs=1))
            kvpool = top.enter_context(tc.tile_pool(name="kv", bufs=1))
            xpool = top.enter_context(tc.tile_pool(name="xp", bufs=2))
            qpool = top.enter_context(tc.tile_pool(name="qp", bufs=2))
            aopool = top.enter_context(tc.tile_pool(name="ao", bufs=3))
            rtmp = top.enter_context(tc.tile_pool(name="rt", bufs=2))
            ppool = top.enter_context(tc.tile_pool(name="pp", bufs=8))
            ost = top.enter_context(tc.tile_pool(name="ost", bufs=5))
            stage = top.enter_context(tc.tile_pool(name="stage", bufs=2))
            stg1 = top.enter_context(tc.tile_pool(name="stg1", bufs=1))
            # PSUM: exactly 8 banks
            acc = top.enter_context(tc.tile_pool(name="acc", bufs=2, space="PSUM"))
            sps = top.enter_context(tc.tile_pool(name="sps", bufs=3, space="PSUM"))
            obp = top.enter_context(tc.tile_pool(name="obp", bufs=2, space="PSUM"))
            smp = top.enter_context(tc.tile_pool(name="smp", bufs=1, space="PSUM"))

            # ---- static tiles ----
            wvs = wpool.tile([P, KO, DPC], bf16)
            wqs = wpool.tile([P, H, KO, HD], bf16)
            wks = wpool.tile([P, H, KO, HD], bf16)
            wot = wpool.tile([P, H, HID], bf16)
            cosT = const.tile([P, S], f32)
            sinT = const.tile([P, S], f32)
            bmt = const.tile([P, 4, 512], bf16)
            zb = const.tile([P, 1], f32)
            ones_col = const.tile([P, 1], bf16)
            kt = kvpool.tile([P, H, S], bf16)        # K^T, RoPE'd, all chunks
            vsb = kvpool.tile([P, ST, H, HD], bf16)  # V natural layout

            nc.vector.memset(zb[:], 0.0)
            nc.vector.memset(ones_col[:], 1.0)
            # fixed softmax-sum bank: only rows 0/32/64/96 are ever written
            # (by the col-group sum matmuls); the rest stays zero so the
            # 128-channel all-reduce combine sees sum + zeros
            sm_fx = smp.tile([P, 512], f32, tag="sm", name="sm")
            nc.vector.memset(sm_fx[:], 0.0)

            # ---- load order tuned for the ko-pipelined chunk-0 V-proj:
            # wv ko-quarters and per-ko x0 slices interleaved on both queues,
            # then wk (sync) / cos+sin+wq (scalar), wot last ----
            xs_c = [xpool.tile([P, KO, 512], bf16, tag="xs", name=f"xs{c}")
                    for c in range(SC)]
            nc.sync.dma_start(wvs[:, 0:4], wv[:, 0:4])
            nc.scalar.dma_start(wvs[:, 4:8], wv[:, 4:8])
            nc.gpsimd.dma_start(xs_c[0][:, 0], xt[:, 0, 0:512])
            nc.gpsimd.dma_start(xs_c[0][:, 1], xt[:, 1, 0:512])
            nc.sync.dma_start(wvs[:, 8:12], wv[:, 8:12])
            nc.scalar.dma_start(wvs[:, 12:16], wv[:, 12:16])
            for ko in range(2, KO):
                eng = (nc.sync, nc.scalar, nc.gpsimd)[ko % 3]
                eng.dma_start(xs_c[0][:, ko], xt[:, ko, 0:512])
            for h in range(H):
                nc.sync.dma_start(wks[:, h], wk[h])
            nc.scalar.dma_start(cosT[:], cosf[:])
            nc.scalar.dma_start(sinT[:], sinf[:])
            for h in range(H):
                nc.scalar.dma_start(wqs[:, h], wq[h])
            nc.scalar.dma_start(bmt[:], bmask[:])
            nc.sync.dma_start(wot[:], wo[:])

            qt_c = [None] * SC
            aot_c = [None] * SC

            # ---------- filler item builders (each item: emit ~1 matmul) ----
            def vproj_items(sc):
                items = []
                box = {}
                for st4 in range(4):
                    st = sc * 4 + st4
                    for ko in range(KO):
                        def mk(st=st, st4=st4, ko=ko):
                            if ko == 0:
                                box['ps'] = acc.tile([P, DPC], f32, tag="acc", name="vps")
                            nc.tensor.matmul(
                                box['ps'][:],
                                xs_c[sc][:, ko, st4 * P:(st4 + 1) * P],
                                wvs[:, ko],
                                start=(ko == 0),
                                stop=(ko == KO - 1),
                            )
                            if ko == KO - 1:
                                nc.vector.tensor_copy(
                                    vsb[:, st],
                                    box['ps'].rearrange("p (h d) -> p h d", h=H),
                                )
                        items.append(mk)
                return items

            def qkproj_items(sc, ws, dst, dst_sl):
                # dst[dst_sl(h)] <- RoPE(ws[h].T @ x_chunk) in bf16
                ssl = slice(sc * 512, (sc + 1) * 512)
                items = []
                box = {}
                for h in range(H):
                    for ko in range(KO):
                        def mk(h=h, ko=ko):
                            if ko == 0:
                                box['ps'] = acc.tile([P, 512], f32, tag="acc", name="qkps")
                            ps = box['ps']
                            nc.tensor.matmul(
                                ps[:],
                                ws[:, h, ko],
                                xs_c[sc][:, ko],
                                start=(ko == 0),
                                stop=(ko == KO - 1),
                            )
                            if ko == KO - 1:
                                # RoPE eviction; sinT pre-signed (-sin top)
                                t0 = rtmp.tile([P, 512], f32, tag="t0", name="t0")
                                c0 = rtmp.tile([P, 512], f32, tag="c0", name="c0")
                                nc.vector.tensor_mul(
                                    t0[0:64], ps[64:128], sinT[0:64, ssl])
                                nc.vector.tensor_mul(
                                    t0[64:128], ps[0:64], sinT[64:128, ssl])
                                nc.vector.tensor_mul(c0[:], ps[:], cosT[:, ssl])
                                nc.vector.tensor_add(dst[dst_sl(h)], c0[:], t0[:])
                        items.append(mk)
                return items

            def oproj_items(cc, alt_out=False):
                aot = aot_c[cc]
                items = []
                box = {}
                for st4 in range(4):
                    st = cc * 4 + st4
                    for nch in range(4):
                        for dc in range(H):
                            def mk(st=st, st4=st4, nch=nch, dc=dc):
                                if dc == 0:
                                    box['ps'] = acc.tile([P, 512], f32, tag="acc", name="ops")
                                pso = box['ps']
                                nc.tensor.matmul(
                                    pso[:],
                                    aot[:, dc, st4 * P:(st4 + 1) * P],
                                    wot[:, dc, nch * 512:(nch + 1) * 512],
                                    start=(dc == 0),
                                    stop=(dc == H - 1),
                                )
                                if dc == H - 1:
                                    ob2 = ost.tile([P, 512], f32, tag="ob2", name="ob2")
                                    # evict on ACT: the DVE is the loaded
                                    # engine (RoPE+masks), scalar has slack
                                    nc.scalar.activation(
                                        ob2[:], pso[:], AF.Copy)
                                    eng = nc.scalar if (alt_out and (st4 * 4 + nch) % 2) else nc.sync
                                    eng.dma_start(
                                        out3[:, st, nch * 512:(nch + 1) * 512],
                                        ob2[:],
                                    )
                            items.append(mk)
                return items

            def mix_filler(a, b):
                """Proportionally merge list b into list a."""
                if not a:
                    return list(b)
                if not b:
                    return list(a)
                out = []
                ia = ib = 0
                while ia < len(a) or ib < len(b):
                    if ib * len(a) <= ia * len(b) and ib < len(b):
                        out.append(b[ib]); ib += 1
                    elif ia < len(a):
                        out.append(a[ia]); ia += 1
                    else:
                        out.append(b[ib]); ib += 1
                return out

            # ---------- attention ----------
            def attention_tiles(sc):
                """Return a list of per-tile emitters; each emits scores+exp
                (+mask), appends to a pend queue, and flushes P@V/sum for the
                tile LOOK back (plus the normalize chain at head end)."""
                nt = 4 * (sc + 1)
                qc = qt_c[sc]
                aot = aot_c[sc]
                # dense tiles first: every sum col-group's start=True write is
                # then full-width [0:512], so no stale PSUM columns survive
                # into the 4-row combine.  sc==0 is all-diagonal: plain
                # per-tile sums into row 0 there.
                t_order = list(range(0, 4 * sc)) + list(range(4 * sc, nt))
                pend = []
                sq = []

                def flush_one():
                    pt, off, ti, t, h, ob, sm = pend.pop(0)
                    nc.tensor.matmul(
                        ob[:, off:512], vsb[:, t, h], pt[:, off:512],
                        start=(ti == 0), stop=(ti == nt - 1),
                    )
                    if sc == 0:
                        nc.tensor.matmul(
                            sm[0:1, off:512], ones_col[:], pt[:, off:512],
                            start=(ti == 0), stop=(ti == nt - 1),
                        )
                    else:
                        sq.append((pt, off, ti))
                        if len(sq) == 4:
                            # softmax sums: 4 concurrent M=1 matmuls in
                            # distinct 32-aligned column groups of the PE
                            # array
                            for (qpt, qoff, qti) in sq:
                                j = qti % 4
                                nc.tensor.matmul(
                                    sm[32 * j:32 * j + 1, qoff:512],
                                    ones_col[:], qpt[:, qoff:512],
                                    start=(qti < 4), stop=(qti >= nt - 4),
                                    tile_position=(0, 32 * j),
                                )
                            sq.clear()
                    if ti == nt - 1:
                        # combine rows 0/32/64/96 (rest of the bank is zero)
                        # via a 128-channel all-reduce, which also serves as
                        # the partition broadcast of 1/Z
                        zsb = stg1.tile([P, 512], f32, tag="zsb", name="zsb")
                        nc.scalar.activation(zsb[:], sm[:], AF.Copy)
                        zr = stg1.tile([P, 512], f32, tag="zr", name="zr")
                        nc.gpsimd.partition_all_reduce(
                            zr[:], zsb[:], channels=P,
                            reduce_op=bass_isa.ReduceOp.add)
                        zrcp = stg1.tile([P, 512], f32, tag="zrcp",
                                         name="zrcp")
                        nc.vector.reciprocal_approx_fast(zrcp[:], zr[:])
                        nc.vector.tensor_mul(aot[:, h], ob[:], zrcp[:])

                items = []
                box = {}
                for h in range(H):
                    for ti, t in enumerate(t_order):
                        def mk(h=h, ti=ti, t=t):
                            if ti == 0:
                                box['ob'] = obp.tile([P, 512], f32, tag="ob", name="ob")
                                box['sm'] = sm_fx
                            r = t - 4 * sc
                            off = P * max(r, 0)
                            ps = sps.tile([P, 512], f32, tag="s", name="sco")
                            nc.tensor.matmul(
                                ps[:, off:512],
                                kt[:, h, t * P:(t + 1) * P],
                                qc[:, h, off:512],
                                start=True, stop=True,
                            )
                            pt = ppool.tile([P, 512], bf16, tag="pt", name="pt")
                            nc.scalar.activation(
                                pt[:, off:512], ps[:, off:512], AF.Exp,
                                bias=zb[:], scale=SCALE,
                            )
                            if r >= 0:
                                nc.vector.tensor_mul(
                                    pt[:, off:512], pt[:, off:512],
                                    bmt[:, r, off:512],
                                )
                            pend.append(
                                (pt, off, ti, t, h, box['ob'], box['sm']))
                            if len(pend) > LOOK:
                                flush_one()
                        items.append(mk)

                def drain():
                    while pend:
                        flush_one()
                return items, drain

            # ---------- main schedule ----------
            for sc in range(SC):
                # prefetch next x chunk; sync queue only -- scalar must stay
                # on the exp stream and gpsimd must stay free for the
                # normalize all-reduce (SWDGE descriptor gen there blocks it)
                if sc + 1 < SC:
                    for ko in range(KO):
                        nc.sync.dma_start(
                            xs_c[sc + 1][:, ko],
                            xt[:, ko, (sc + 1) * 512:(sc + 2) * 512],
                        )
                aot_c[sc] = aopool.tile([P, H, 512], bf16, tag="aot",
                                        name=f"aot{sc}")
                if sc == 0:
                    # no preceding attention to interleave with: emit densely
                    qt_c[0] = qpool.tile([P, H, 512], bf16, tag="qt", name="qt0")
                    for it in vproj_items(0):
                        it()
                    for it in qkproj_items(
                            0, wks, kt,
                            lambda h: (slice(None), h, slice(0, 512))):
                        it()
                    for it in qkproj_items(
                            0, wqs, qt_c[0],
                            lambda h: (slice(None), h, slice(None))):
                        it()

                # filler: o_proj work is deferred TWO chunks so the last
                # chunk (which has no next-chunk projections) still gets two
                # o_proj chunks' worth of PE filler to hide exp latency.
                # o_proj items are spread proportionally among proj items so
                # their output DMAs don't burst the sync queue.
                oproj_f = []
                proj_f = []
                if sc >= 2:
                    oproj_f += oproj_items(sc - 2)
                if sc == SC - 1:
                    oproj_f += oproj_items(sc - 1, alt_out=True)
                if sc + 1 < SC:
                    nsc = sc + 1
                    qt_c[nsc] = qpool.tile([P, H, 512], bf16, tag="qt",
                                           name=f"qt{nsc}")
                    proj_f += vproj_items(nsc)
                    proj_f += qkproj_items(
                        nsc, wks, kt,
                        lambda h, _s=nsc: (slice(None), h,
                                           slice(_s * 512, (_s + 1) * 512)))
                    proj_f += qkproj_items(
                        nsc, wqs, qt_c[nsc],
                        lambda h: (slice(None), h, slice(None)))
                filler = mix_filler(proj_f, oproj_f)

                tiles, drain = attention_tiles(sc)
                # delay filler that reads the just-prefetched x chunk by a
                # few tiles so the DMA has time to land
                lead = 6 if (proj_f and sc >= 1) else 0
                step = len(filler) / max(1, len(tiles) - lead)
                fi_target = -lead * step
                fi = 0
                for it in tiles:
                    it()
                    fi_target += step
                    while fi < int(fi_target):
                        filler[fi]()
                        fi += 1
                drain()
                while fi < len(filler):
                    filler[fi]()
                    fi += 1

            for it in oproj_items(SC - 1):
                it()

    nc.compile()
    return nc


def _host_prep(hidden_states, position_ids, Wq, Wk, Wv, Wo):
    """Build the 8 per-core input maps (bf16 operands)."""
    inv_freq = 1.0 / (10000.0 ** (np.arange(0, HD, 2, dtype=np.float32) / HD))
    t = np.arange(S, dtype=np.float32)
    freqs = np.outer(t, inv_freq).astype(np.float32)  # [S, 64]

    bm = np.empty((P, 4, 512), dtype=np.float32)
    i = np.arange(P)[:, None, None]
    r = np.arange(4)[None, :, None]
    j = np.arange(512)[None, None, :]
    bm[:] = np.where(i + P * r <= j, 1.0, 0.0)
    bm = bm.astype(ml_dtypes.bfloat16)

    in_maps = []
    per_batch = []
    for b in range(B):
        xT = np.ascontiguousarray(hidden_states[b].T)  # [HID, S]
        xt_sw = np.ascontiguousarray(
            xT.reshape(KO, P, S).transpose(1, 0, 2)
        ).astype(ml_dtypes.bfloat16)  # [P, KO, S]
        fp = freqs[position_ids[b]]  # [S, 64]
        ch = np.cos(fp).T            # [64, S]
        sh = np.sin(fp).T
        cosf = np.ascontiguousarray(np.concatenate([ch, ch], axis=0))   # [128, S]
        sinf = np.ascontiguousarray(np.concatenate([-sh, sh], axis=0))  # signed
        per_batch.append((xt_sw, cosf, sinf))

    for core in range(8):
        b, hg = core // 4, core % 4
        sl = slice(hg * DPC, (hg + 1) * DPC)
        xt_sw, cosf, sinf = per_batch[b]
        wq_sw = np.ascontiguousarray(
            Wq[sl].T.reshape(KO, P, H, HD).transpose(2, 1, 0, 3)
        ).astype(ml_dtypes.bfloat16)  # [H, P, KO, HD]
        wk_sw = np.ascontiguousarray(
            Wk[sl].T.reshape(KO, P, H, HD).transpose(2, 1, 0, 3)
        ).astype(ml_dtypes.bfloat16)
        wv_sw = np.ascontiguousarray(
            Wv[sl].T.reshape(KO, P, DPC).transpose(1, 0, 2)
        ).astype(ml_dtypes.bfloat16)  # [P, KO, DPC]
        wo_sw = np.ascontiguousarray(
            Wo[:, sl].T.reshape(H, HD, HID).transpose(1, 0, 2)
        ).astype(ml_dtypes.bfloat16)  # [P, H, HID]
        in_maps.append({
            "xt": xt_sw, "wq": wq_sw, "wk": wk_sw, "wv": wv_sw, "wo": wo_sw,
            "cosf": cosf, "sinf": sinf, "bmask": bm,
        })
    return in_maps


def kernel(hidden_states, attention_mask, position_ids, Wq, Wk, Wv, Wo,
           _trace=False, _trace_kwargs=None):
    global _CACHED_NC
    hidden_states = np.asarray(hidden_states, dtype=np.float32)
    position_ids = np.asarray(position_ids)
    Wq, Wk, Wv, Wo = (np.asarray(w, dtype=np.float32) for w in (Wq, Wk, Wv, Wo))

    if _CACHED_NC is None:
        _CACHED_NC = build_nc()
    nc = _CACHED_NC

    in_maps = _host_prep(hidden_states, position_ids, Wq, Wk, Wv, Wo)
    res = run_bass_kernel_spmd(
        nc, in_maps, list(range(8)), trace=_trace, **(_trace_kwargs or {})
    )

    out = np.empty((B, S, HID), dtype=np.float32)
    for b in range(B):
        acc = res.results[b * 4]["out_p"].astype(np.float32)
        for hg in range(1, 4):
            acc = acc + res.results[b * 4 + hg]["out_p"]
        out[b] = acc
    if _trace:
        return out, res
    return out
